# revision 12
# baseline (speedup 1.0000x reference)
"""Causal single-head attention (B=4, S=2048, d=1024) on 8 TRN2 NeuronCores.

Sharding (uniform single program): core c -> batch b = c//2, subset
s = c%2. Per batch, the 16 query blocks of 128 rows are split into
quads t=0..3; core (b,s) owns blocks {4t+2s, 4t+2s+1}. Every core runs
the identical instruction stream (padded causal limit (t+1)*512 per
quad); the true causal boundary comes from per-core 0/1 mask tiles
supplied as input data.

K/V projections are tensor-parallel within each core pair: core (b,s)
computes the d_out-half s of kT and v for the whole batch; halves are
exchanged with pairwise AllGathers ([[0,1],[2,3],[4,5],[6,7]]).

v3 schedule: K projection streams first over resident 512-key x
chunks; its half is AllGathered in two 1 MB groups, triggered as each
half of the keys completes. V projection follows, reusing the resident
x chunks (xT loaded once), with two more 1 MB gathers. Q projection
runs while the V gathers drain the link. Attention then runs per quad
in DESCENDING t order with scores (A) and AV (B) interleaved, so the
kernel tail is the shortest quad.

Compute (bf16 operands, fp32 PSUM accumulate):
  scoresT[k,q] = kt.T-slices @ qT-group, p = exp(scoresT)
  (no max subtraction: |scores| <= ~2), mask, then
  out[q,:] = (pT.T @ v) / (pT.T @ ones)  -- row sums via ones-matmul.
"""
import sys

sys.path.insert(0, "/opt/trn_rl_repo")

import ml_dtypes
import numpy as np

import concourse.bass as bass  # noqa: F401
import concourse.mybir as mybir
import concourse.tile as tile
from concourse import bacc
from concourse.bass_utils import run_bass_kernel_spmd

B, S, D = 4, 2048, 1024
DC = D // 128          # 8 contraction chunks
NKB = S // 128         # 16 key blocks
SCALE = 1.0 / float(np.sqrt(D))
F32 = mybir.dt.float32
BF = mybir.dt.bfloat16
EXP = mybir.ActivationFunctionType.Exp
GROUPS = [[0, 1], [2, 3], [4, 5], [6, 7]]

_cache = {}


def build_nc():
    nc = bacc.Bacc("TRN2", target_bir_lowering=False, debug=False, num_devices=8)
    # all inputs partition-major: [128, dc, cols]
    xT = nc.dram_tensor("xT", [128, DC, S], BF, kind="ExternalInput")
    xTq = nc.dram_tensor("xTq", [128, DC, 1024], BF, kind="ExternalInput")
    WqT = nc.dram_tensor("WqT", [128, DC, D], BF, kind="ExternalInput")
    WkTh = nc.dram_tensor("WkTh", [128, DC, 512], BF, kind="ExternalInput")
    WvTh = nc.dram_tensor("WvTh", [128, DC, 512], BF, kind="ExternalInput")
    masks = nc.dram_tensor("masks", [128, 4, 256], BF, kind="ExternalInput")
    out = nc.dram_tensor("out", [1024, D], F32, kind="ExternalOutput")
    # collective buffers, group-major: g covers keys [1024g, 1024g+1024)
    kg_in = nc.dram_tensor("kg_in", [2, 128, 4, 1024], BF)
    kg_out = nc.dram_tensor("kg_out", [2, 2, 128, 4, 1024], BF)
    vg_in = nc.dram_tensor("vg_in", [2, 128, 8, 512], BF)
    vg_out = nc.dram_tensor("vg_out", [2, 2, 128, 8, 512], BF)

    with tile.TileContext(nc) as tc:
        with (
            tc.tile_pool(name="w", bufs=1) as wp,
            tc.tile_pool(name="per", bufs=1) as per,
            tc.tile_pool(name="px", bufs=4) as pxp,
            tc.tile_pool(name="stg", bufs=2) as stg,
            tc.tile_pool(name="pt", bufs=28) as ptp,
            tc.tile_pool(name="ot", bufs=2) as otp,
            tc.tile_pool(name="sml", bufs=4) as smlp,
            tc.tile_pool(name="mix", bufs=5, space="PSUM") as mixp,
            tc.tile_pool(name="psav", bufs=3, space="PSUM") as psavp,
        ):
            # ---------------- consts + persistent ----------------
            kt = per.tile([128, DC, S], BF)        # kT: [d_out, 2048]
            vv = per.tile([128, 2, NKB, 512], BF)  # v: [2048, (rank0|rank1) 512]
            qt = per.tile([128, DC, 1024], BF)     # qT: [d_out, 1024]
            zeros_f = per.tile([128, 2], F32)
            ones = per.tile([128, 2], BF)
            maskt = per.tile([128, 4, 256], BF)
            nc.vector.memset(zeros_f, 0.0)
            # exp(0)=1 -> also preloads the ACT exp table long before attention
            nc.scalar.activation(ones, zeros_f, EXP)

            wk = wp.tile([128, DC, 512], BF)
            wv = wp.tile([128, DC, 512], BF)
            wq = wp.tile([128, DC, D], BF)
            # sync queue: wk first (split per dc-pair so the first matmul
            # chain starts as soon as its slices land), the resident x
            # chunks, then the Q-projection weights.
            for dp in range(4):
                nc.sync.dma_start(
                    out=wk[:, 2 * dp:2 * dp + 2, :],
                    in_=WkTh[:, 2 * dp:2 * dp + 2, :],
                )
            xks = []
            for sc in range(4):
                xk = pxp.tile([128, DC, 512], BF, tag="xs", name=f"xk_{sc}")
                if sc == 0:
                    for dp in range(4):
                        nc.sync.dma_start(
                            out=xk[:, 2 * dp:2 * dp + 2, :],
                            in_=xT[:, 2 * dp:2 * dp + 2, 0:512],
                        )
                else:
                    nc.sync.dma_start(
                        out=xk, in_=xT[:, :, sc * 512:(sc + 1) * 512]
                    )
                xks.append(xk)
            nc.sync.dma_start(out=wq, in_=WqT[:])
            # scalar queue: V weights + masks up front
            nc.scalar.dma_start(out=wv, in_=WvTh[:])
            nc.scalar.dma_start(out=maskt, in_=masks[:])

            # -------- K half-projection, gathered in two 1MB groups --------
            for g in range(2):
                kgs = stg.tile([128, 4, 1024], BF, tag="kgs", name=f"kgs_{g}")
                for scl in range(2):
                    xk = xks[2 * g + scl]
                    for ocl in range(4):
                        ps = mixp.tile([128, 512], F32, tag="mix")
                        for dc in range(DC):
                            nc.tensor.matmul(
                                ps,
                                lhsT=wk[:, dc, ocl * 128:(ocl + 1) * 128],
                                rhs=xk[:, dc, :],
                                start=(dc == 0),
                                stop=(dc == DC - 1),
                            )
                        nc.vector.tensor_copy(
                            kgs[:, ocl, scl * 512:(scl + 1) * 512], ps
                        )
                nc.scalar.dma_start(out=kg_in[g], in_=kgs)
                nc.gpsimd.collective_compute(
                    "AllGather",
                    mybir.AluOpType.bypass,
                    replica_groups=GROUPS,
                    ins=[kg_in[g]],
                    outs=[kg_out[g]],
                )

            # -------- V half-projection (reuses resident x), two gathers ----
            for g in range(2):
                vgs = stg.tile([128, 8, 512], BF, tag="vgs", name=f"vgs_{g}")
                for scl in range(2):
                    xk = xks[2 * g + scl]
                    for sb in range(4):
                        ps = mixp.tile([128, 512], F32, tag="mix",
                                       name=f"ps2_{g}_{scl}_{sb}")
                        for dc in range(DC):
                            nc.tensor.matmul(
                                ps,
                                lhsT=xk[:, dc, sb * 128:(sb + 1) * 128],
                                rhs=wv[:, dc, :],
                                start=(dc == 0),
                                stop=(dc == DC - 1),
                            )
                        nc.vector.tensor_copy(vgs[:, scl * 4 + sb, :], ps)
                nc.scalar.dma_start(out=vg_in[g], in_=vgs)
                nc.gpsimd.collective_compute(
                    "AllGather",
                    mybir.AluOpType.bypass,
                    replica_groups=GROUPS,
                    ins=[vg_in[g]],
                    outs=[vg_out[g]],
                )

            # gathered K groups -> SBUF (sync queue, behind wq/xq)
            for g in range(2):
                for r in range(2):
                    nc.sync.dma_start(
                        out=kt[:, r * 4:(r + 1) * 4, g * 1024:(g + 1) * 1024],
                        in_=kg_out[g, r],
                    )
            # gathered V groups -> SBUF (scalar queue)
            for g in range(2):
                for r in range(2):
                    nc.scalar.dma_start(
                        out=vv[:, r, 8 * g:8 * g + 8, :], in_=vg_out[g, r]
                    )

            # Q-projection activations ride the freed x-chunk slots
            xqs = []
            for h in range(2):
                xqh = pxp.tile([128, DC, 512], BF, tag="xs", name=f"xq_{h}")
                nc.sync.dma_start(out=xqh, in_=xTq[:, :, h * 512:(h + 1) * 512])
                xqs.append(xqh)

            # -------- Q projection -> qt --------
            for oc in range(8):
                pss = [
                    mixp.tile([128, 512], F32, tag="mix", name=f"ps0_{oc}_{i}")
                    for i in range(2)
                ]
                for dc in range(DC):
                    for sc in range(2):
                        nc.tensor.matmul(
                            pss[sc],
                            lhsT=wq[:, dc, oc * 128:(oc + 1) * 128],
                            rhs=xqs[sc][:, dc, :],
                            start=(dc == 0),
                            stop=(dc == DC - 1),
                        )
                for sc in range(2):
                    nc.vector.tensor_copy(
                        qt[:, oc, sc * 512:(sc + 1) * 512], pss[sc]
                    )

            # ---------------- attention ----------------
            all_pts = {}

            def emit_scores(t):
                L = 4 * t + 4
                pts = []
                for kb in range(L):
                    ps = mixp.tile([128, 512], F32, tag="mix",
                                   name=f"sc_{t}_{kb}")
                    for dc in range(DC):
                        nc.tensor.matmul(
                            ps[:, 0:256],
                            lhsT=kt[:, dc, kb * 128:(kb + 1) * 128],
                            rhs=qt[:, dc, t * 256:(t + 1) * 256],
                            start=(dc == 0),
                            stop=(dc == DC - 1),
                        )
                    pt = ptp.tile([128, 256], BF, tag="pt", name=f"pt_{t}_{kb}")
                    nc.scalar.activation(pt, ps[:, 0:256], EXP)
                    kbr = kb - 4 * t
                    if kbr >= 0:
                        nc.vector.tensor_mul(pt, pt, maskt[:, kbr, :])
                    pts.append(pt)
                all_pts[t] = pts

            def emit_av(t):
                L = 4 * t + 4
                pts = all_pts.pop(t)
                recs, ots = [], []
                for j in range(2):
                    qsl = slice(j * 128, (j + 1) * 128)
                    lps = psavp.tile([128, 2], F32, tag="psav", name=f"l_{t}_{j}")
                    for kb in range(L):
                        nc.tensor.matmul(
                            lps,
                            lhsT=pts[kb][:, qsl],
                            rhs=ones,
                            start=(kb == 0),
                            stop=(kb == L - 1),
                        )
                    rec = smlp.tile([128, 1], F32, tag="rec")
                    nc.vector.reciprocal(rec, lps[:, 0:1])
                    recs.append(rec)
                    ots.append(otp.tile([128, D], F32, tag="ot", name=f"ot{t}{j}"))
                for j in range(2):
                    qsl = slice(j * 128, (j + 1) * 128)
                    for oh in range(2):
                        avp = psavp.tile([128, 512], F32, tag="psav",
                                         name=f"av_{t}_{j}_{oh}")
                        for kb in range(L):
                            nc.tensor.matmul(
                                avp,
                                lhsT=pts[kb][:, qsl],
                                rhs=vv[:, oh, kb, :],
                                start=(kb == 0),
                                stop=(kb == L - 1),
                            )
                        nc.vector.tensor_scalar_mul(
                            ots[j][:, oh * 512:(oh + 1) * 512], avp, recs[j]
                        )
                        nc.sync.dma_start(
                            out=out[t * 256 + j * 128: t * 256 + (j + 1) * 128,
                                    oh * 512:(oh + 1) * 512],
                            in_=ots[j][:, oh * 512:(oh + 1) * 512],
                        )

            # B3 sits after A2 to give the trailing V gather extra slack;
            # the kernel still ends on the shortest quad's chain (B0).
            emit_scores(3)
            emit_scores(2)
            emit_av(3)
            emit_scores(1)
            emit_av(2)
            emit_scores(0)
            emit_av(1)
            emit_av(0)
    nc.compile()
    return nc


def _query_cols(sub):
    return np.concatenate(
        [
            np.arange((4 * t + 2 * sub) * 128, (4 * t + 2 * sub + 2) * 128)
            for t in range(4)
        ]
    )


def _masks(sub):
    m = np.zeros((4, 128, 256), np.float32)
    p = np.arange(128)[:, None]
    j = np.arange(256)[None, :]
    qoff = (2 * sub + j // 128) * 128 + j % 128
    for kbr in range(4):
        m[kbr] = (kbr * 128 + p <= qoff).astype(np.float32)
    return np.ascontiguousarray(m.transpose(1, 0, 2))  # -> [128, 4, 256]


def _pmaj(a):
    """[dc*128, cols] -> partition-major [128, dc, cols]."""
    d, cols = a.shape
    return np.ascontiguousarray(a.reshape(d // 128, 128, cols).transpose(1, 0, 2))


def kernel(x, Wq, Wk, Wv, _trace=False):
    if "nc" not in _cache:
        _cache["nc"] = build_nc()
    nc = _cache["nc"]

    bf = ml_dtypes.bfloat16
    x = np.asarray(x, dtype=np.float32)
    WqT = _pmaj((np.asarray(Wq, np.float32).T * np.float32(SCALE)).astype(bf))
    WkT = np.asarray(Wk, np.float32).T.astype(bf)
    WvT = np.asarray(Wv, np.float32).T.astype(bf)

    in_maps = []
    for c in range(8):
        b, sub = c // 2, c % 2
        xT = x[b].T.astype(bf)
        in_maps.append(
            {
                "xT": _pmaj(xT),
                "xTq": _pmaj(np.ascontiguousarray(xT[:, _query_cols(sub)])),
                "WqT": WqT,
                "WkTh": _pmaj(WkT[:, sub * 512:(sub + 1) * 512]),
                "WvTh": _pmaj(WvT[:, sub * 512:(sub + 1) * 512]),
                "masks": _masks(sub).astype(bf),
            }
        )

    res = run_bass_kernel_spmd(
        nc, in_maps, core_ids=list(range(8)), trace=_trace
    )
    full = np.empty((B, S, D), np.float32)
    for c in range(8):
        b, sub = c // 2, c % 2
        full[b, _query_cols(sub)] = res.results[c]["out"]
    if _trace:
        _cache["last_result"] = res
    return full
